# revision 46
# baseline (speedup 1.0000x reference)
"""Multi-head attention (B=4, S=1024, H=1024, 16 heads) on 8 trn2 cores.

Sharding: 8 shards = (batch b in 0..3) x (head-half hf in 0..1).
Each core computes attention for 8 heads of one batch and a partial
output projection (row-parallel Wo); host sums the two partials per batch.

Per-core pipeline (matmuls bf16, PSUM fp32, output fp32):
  - inputs as [128, ktile, *] slabs so one or two DMAs load each tensor
    (HWDGE descriptor time, ~630ns each, would otherwise serialize startup)
  - QT/KT d-major bf16 slabs; V token-major bf16 with ones column per head
  - logitsT[k, q] per head via lhsT=KT tile (K=64), exp on ACT with
    per-partition bias fused (logits are O(+-9): fp32 exp, no max-sub)
  - attn@V reoriented: lhsT = exp tile [k, qtile], rhs = V_aug [k, 65]
    -> psum [q, 65] accumulated over k-tiles; col 64 = softmax denominator.
    Halves attn@V PE cycles vs moving the q dimension.
  - normalize: DVE reciprocal + per-partition scalar mul into [q, 128]
    head-pair tiles; DMA-XBAR transpose (free, idle DMA engines) back to
    [dq, tok] at-slabs for the output projection
  - Wo tail: each [tok-tile, hid-half] chunk accumulates all 4 dq-pairs in
    one PSUM group; psum->SBUF copies alternate DVE/ACT (ACT idle at tail);
    DMA per chunk overlaps the remaining matmuls
  - software pipelined: head h's attn@V rides inside head h+1's logits/exp
    stream; V bursts ride inside head 0's slots (wv arrives last)
"""

import numpy as np
import ml_dtypes

import concourse.bass as bass
import concourse.tile as tile
from concourse import bacc, mybir
from concourse import bass_utils

F32 = mybir.dt.float32
BF16 = mybir.dt.bfloat16
EXP = mybir.ActivationFunctionType.Exp

S = 1024  # sequence length (tokens)
HID = 1024  # model hidden
DQ = 512  # per-core projected dim (8 heads x 64)
NHL = 8  # local heads per core
DH = 64  # head depth
NK = HID // 128  # 8 contraction tiles over hidden
P = 128
N_CORES = 8

MM_DT = BF16

_CACHED_NC = None


def build_program(unroll=1):
    nc = bacc.Bacc("TRN2", target_bir_lowering=False, debug=False)
    # slab layouts [128, ktile, *]: host pre-rearranges; 1-2 DMAs per tensor
    xt = nc.dram_tensor("xt", [P, NK, S], BF16, kind="ExternalInput").ap()
    yt = nc.dram_tensor("yt", [P, NK, S], BF16, kind="ExternalInput").ap()
    wq = nc.dram_tensor("wq", [P, NK, DQ], BF16, kind="ExternalInput").ap()
    wk = nc.dram_tensor("wk", [P, NK, DQ], BF16, kind="ExternalInput").ap()
    wv = nc.dram_tensor("wv", [P, NK, DQ], BF16, kind="ExternalInput").ap()
    wo = nc.dram_tensor("wo", [P, 4, HID], BF16, kind="ExternalInput").ap()
    biasd = nc.dram_tensor("biasd", [P, NK], F32, kind="ExternalInput").ap()
    onesd = nc.dram_tensor("onesd", [P, NHL], BF16, kind="ExternalInput").ap()
    # bf16 partials: host upcasts and sums the two head-half partials
    out = nc.dram_tensor("out", [S, HID], BF16, kind="ExternalOutput").ap()

    with tile.TileContext(nc) as tc:
        for _ in range(unroll):
            emit_kernel(tc, out, xt, yt, wq, wk, wv, wo, biasd, onesd)
    nc.compile()
    return nc


def emit_kernel(tc, out, xt, yt, wq, wk, wv, wo, biasd, onesd):
    nc = tc.nc
    with (
        tc.tile_pool(name="wpool", bufs=1) as wpool,
        tc.tile_pool(name="qkv", bufs=1) as qkvpool,
        tc.tile_pool(name="atp", bufs=1) as atpool,
        tc.tile_pool(name="xypool", bufs=1) as xypool,
    ):
        # ---- batched slab DMA, critical-path first (exp stream is gated on
        # the Q then K projections): xt+wq, yt+wk, bias, then wv, wo.
        xt_sl = xypool.tile([P, NK, S], MM_DT, tag="xt", name="xt_sl")
        yt_sl = xypool.tile([P, NK, S], MM_DT, tag="yt", name="yt_sl")
        wq_sl = wpool.tile([P, NK, DQ], MM_DT, tag="wq", name="wq_sl")
        wk_sl = wpool.tile([P, NK, DQ], MM_DT, tag="wk", name="wk_sl")
        wv_sl = wpool.tile([P, NK, DQ], MM_DT, tag="wv", name="wv_sl")
        wo_sl = wpool.tile([P, 4, HID], MM_DT, tag="wo", name="wo_sl")
        # ALL input DMAs on one ring, strict priority order: a second ring
        # would drain its (smaller) queue ahead and steal serial DMA-engine
        # bandwidth from the critical xt/wq/yt/wk stream. First chunks are
        # single k-tiles so the first matmul starts sooner.
        for a, b in ((0, 1), (1, 4), (4, 8)):
            nc.sync.dma_start(xt_sl[:, a:b, :], xt[:, a:b, :])
            nc.sync.dma_start(wq_sl[:, a:b, :], wq[:, a:b, :])
        for a, b in ((0, 4), (4, 8)):
            nc.sync.dma_start(yt_sl[:, a:b, :], yt[:, a:b, :])
            nc.sync.dma_start(wk_sl[:, a:b, :], wk[:, a:b, :])
        bias_sb = wpool.tile([P, NK], F32, tag="bias")
        nc.sync.dma_start(bias_sb[:], biasd[:])
        vones_sb = wpool.tile([P, NHL], BF16, tag="vones")
        nc.sync.dma_start(vones_sb[:], onesd[:])
        nc.sync.dma_start(wv_sl[:, 0:4, :], wv[:, 0:4, :])
        nc.sync.dma_start(wv_sl[:, 4:8, :], wv[:, 4:8, :])
        nc.sync.dma_start(wo_sl[:], wo[:])

        # ---- persistent slabs ----
        qt_sb = [qkvpool.tile([P, S], MM_DT, tag=f"qt{m}", name=f"qt{m}") for m in range(4)]
        kt_sb = [qkvpool.tile([P, S], MM_DT, tag=f"kt{m}", name=f"kt{m}") for m in range(4)]
        v_sb = [qkvpool.tile([P, NHL * (DH + 1)], MM_DT, tag=f"v{m}", name=f"v{m}") for m in range(8)]
        at_sb = [atpool.tile([P, S], MM_DT, tag=f"at{m}", name=f"at{m}") for m in range(4)]

        # PSUM (8 banks). Prologue: pro 4x[128,512]. Head loop:
        # lg 2x[128,1024]=4, av 2x[128,65]=2, pj 2x[128,512]=2.
        # Tail (all released): wo 6x[128,512].
        # PE p-state warm-up: ~3us of dummy matmuls on a memset tile while
        # the first input DMAs stream, so real matmuls run at full clock
        pp_warm = tc.alloc_tile_pool(name="pp_warm", bufs=2, space="PSUM")
        warm_sb = wpool.tile([P, 512], MM_DT, tag="warm", name="warm")
        nc.vector.memset(warm_sb[:], 0.0)
        for w in range(6):
            wps = pp_warm.tile([P, 512], F32, tag="warm", name="warmps")
            nc.tensor.matmul(wps[:], warm_sb[:, 0:P], warm_sb[:], start=True, stop=True)
        pp_warm.release()
        pp_pro = tc.alloc_tile_pool(name="pp_pro", bufs=4, space="PSUM")

        with (
            tc.tile_pool(name="expp", bufs=40) as exppool,
            tc.tile_pool(name="smallp", bufs=6) as smallpool,
            tc.tile_pool(name="aqp", bufs=2) as aqpool,
            tc.tile_pool(name="stagep", bufs=1) as stagepool,
        ):
            e_tiles = [[None] * NK for _ in range(NHL)]  # e[h][sk]
            aq_tiles = [None] * NK  # per qtile, current pair's [q,128] tile
            stage_sb = [
                stagepool.tile([P, 512], BF16, tag=f"st{g}", name=f"st{g}")
                for g in range(16)
            ]

            # ---- emission helpers ----
            def emit_v_burst(m, pool):
                # V projection for token tile m (token-major, ones appended)
                ps = pool.tile([P, DQ], F32, tag="pj", name="pjv")
                for k in range(NK):
                    nc.tensor.matmul(
                        ps[:],
                        yt_sl[:, k, m * P : (m + 1) * P],
                        wv_sl[:, k, :],
                        start=(k == 0),
                        stop=(k == NK - 1),
                    )
                dst3 = v_sb[m][:].rearrange("p (h c) -> p h c", c=DH + 1)
                src3 = ps[:].rearrange("p (h c) -> p h c", c=DH)
                nc.vector.tensor_copy(dst3[:, :, 0:DH], src3[:, :, :])
                nc.vector.tensor_copy(
                    dst3[:, :, DH : DH + 1],
                    vones_sb[:].rearrange("p (a b) -> p a b", b=1),
                )

            def emit_proj_burst(pair, which, n, pool):
                # one [128dq, 512tok] chunk of the q/k projection for `pair`
                w_sl, src_sl, dst = (
                    (wq_sl, xt_sl, qt_sb) if which == "q" else (wk_sl, yt_sl, kt_sb)
                )
                ps = pool.tile([P, 512], F32, tag="pj", name="pj")
                for k in range(NK):
                    nc.tensor.matmul(
                        ps[:],
                        w_sl[:, k, pair * P : (pair + 1) * P],
                        src_sl[:, k, n * 512 : (n + 1) * 512],
                        start=(k == 0),
                        stop=(k == NK - 1),
                    )
                nc.vector.tensor_copy(dst[pair][:, n * 512 : (n + 1) * 512], ps[:])

            def emit_lg_exp(h, sk):
                pair, hi = divmod(h, 2)
                base = hi * DH
                lg = pp_lg.tile([P, S], F32, tag="lg", name="lg")
                for n in range(2):
                    nc.tensor.matmul(
                        lg[:, n * 512 : (n + 1) * 512],
                        kt_sb[pair][base : base + DH, sk * P : (sk + 1) * P],
                        qt_sb[pair][base : base + DH, n * 512 : (n + 1) * 512],
                        start=True,
                        stop=True,
                    )
                e = exppool.tile([P, S], MM_DT, tag="exp", name="exp")
                nc.scalar.activation(e[:], lg[:], EXP, bias=bias_sb[:, sk : sk + 1])
                return e

            def emit_av(h, qtile, pool=None):
                # attn@V for (head h, query tile qtile): psum [128q, 65]
                pair, hi = divmod(h, 2)
                av = (pool or pp_av).tile([P, DH + 1], F32, tag="av", name="av")
                for sk in range(NK):
                    nc.tensor.matmul(
                        av[:],
                        e_tiles[h][sk][:, qtile * P : (qtile + 1) * P],
                        v_sb[sk][:, h * (DH + 1) : (h + 1) * (DH + 1)],
                        start=(sk == 0),
                        stop=(sk == NK - 1),
                    )
                rc = smallpool.tile([P, 1], F32, tag="rc", name="rc")
                nc.vector.reciprocal(rc[:], av[:, DH : DH + 1])
                if hi == 0:
                    aq = aqpool.tile([P, P], MM_DT, tag=f"aq{qtile}", name=f"aq{qtile}")
                    aq_tiles[qtile] = aq
                else:
                    aq = aq_tiles[qtile]
                nc.vector.tensor_scalar_mul(
                    aq[:, hi * DH : (hi + 1) * DH], av[:, 0:DH], rc[:]
                )
                if hi == 1:
                    nc.sync.dma_start_transpose(
                        at_sb[pair][:, qtile * P : (qtile + 1) * P], aq[:]
                    )

            # ---- prologue: Q projections for ALL pairs (xt/wq arrive
            # first; PE otherwise idles in the DMA-paced window), then K for
            # pair 0 (the gate for the first logits/exp). Full-row N=1024
            # matmuls: one weight load per k-tile.
            for pr in range(4):
                for n in range(2):
                    emit_proj_burst(pr, "q", n, pp_pro)
            for n in range(2):
                emit_proj_burst(0, "k", n, pp_pro)
            pp_pro.release()
            pp_av = tc.alloc_tile_pool(name="pp_av", bufs=2, space="PSUM")
            pp_pj = tc.alloc_tile_pool(name="pp_pj", bufs=2, space="PSUM")
            pp_lg = tc.alloc_tile_pool(name="pp_lg", bufs=2, space="PSUM")

            # ---- software-pipelined head loop ----
            # Per block h: lg/exp stream (the ACT pace-setter) + fillers
            # (remaining K projections, V bursts once wv lands, lagged attn@V
            # groups), balanced so no block's PE work exceeds the ACT pace.
            fillers = {
                (0, 2): lambda: emit_proj_burst(1, "k", 0, pp_pj),
                (0, 5): lambda: emit_proj_burst(1, "k", 1, pp_pj),
                (1, 1): lambda: emit_v_burst(0, pp_pj),
                (1, 3): lambda: emit_v_burst(1, pp_pj),
                (1, 5): lambda: emit_v_burst(2, pp_pj),
                (2, 1): lambda: emit_v_burst(3, pp_pj),
                (2, 4): lambda: emit_v_burst(4, pp_pj),
                (2, 6): lambda: emit_proj_burst(2, "k", 0, pp_pj),
                (3, 1): lambda: emit_v_burst(5, pp_pj),
                (3, 3): lambda: emit_v_burst(6, pp_pj),
                (3, 5): lambda: emit_v_burst(7, pp_pj),
                (3, 7): lambda: emit_proj_burst(2, "k", 1, pp_pj),
                (4, 2): lambda: emit_proj_burst(3, "k", 0, pp_pj),
                (5, 2): lambda: emit_proj_burst(3, "k", 1, pp_pj),
            }

            av_sched = {4: [0], 5: [1], 6: [2, 3], 7: [4, 5]}
            for h in range(NHL):
                for sk in range(NK):
                    e_tiles[h][sk] = emit_lg_exp(h, sk)
                    f = fillers.get((h, sk))
                    if f is not None:
                        f()
                    for avh in av_sched.get(h, ()):
                        emit_av(avh, sk)
            for qtile in range(NK):
                emit_av(6, qtile)
            # ---- tail: av(7) drain on a deeper PSUM pool (3 groups in
            # flight), then Wo: 4-pair PSUM accumulation per output chunk,
            # psum->SBUF copies alternating DVE/ACT, DMA per chunk
            pp_lg.release()
            pp_pj.release()
            pp_av2 = tc.alloc_tile_pool(name="pp_av2", bufs=2, space="PSUM")
            pp_wo = tc.alloc_tile_pool(name="pp_wo", bufs=3, space="PSUM")

            def emit_wo(m, n):
                g = 2 * m + n
                ps = pp_wo.tile([P, 512], F32, tag="wops", name="wops")
                for pair in range(4):
                    nc.tensor.matmul(
                        ps[:],
                        at_sb[pair][:, m * P : (m + 1) * P],
                        wo_sl[:, pair : pair + 1, n * 512 : (n + 1) * 512],
                        start=(pair == 0),
                        stop=(pair == 3),
                    )
                if g % 2 == 0:
                    nc.vector.tensor_copy(stage_sb[g][:], ps[:])
                else:
                    nc.scalar.copy(stage_sb[g][:], ps[:])
                (nc.sync if g % 2 == 0 else nc.scalar).dma_start(
                    out[m * P : (m + 1) * P, n * 512 : (n + 1) * 512], stage_sb[g][:]
                )

            for qtile in range(NK):
                emit_av(NHL - 1, qtile,
                        pool=(pp_av2 if qtile % 2 == 0 else pp_av))
            for qtile in range(NK):
                emit_wo(qtile, 0)
                emit_wo(qtile, 1)
            pp_wo.release()
            pp_av2.release()
            pp_av.release()


def _prep_in_maps(x, y, bias, Wq, Wk, Wv, Wo):
    x = np.asarray(x, dtype=np.float32)
    y = np.asarray(y, dtype=np.float32)
    bias = np.asarray(bias, dtype=np.float32)
    Wq = np.asarray(Wq, dtype=np.float32)
    Wk = np.asarray(Wk, dtype=np.float32)
    Wv = np.asarray(Wv, dtype=np.float32)
    Wo = np.asarray(Wo, dtype=np.float32)
    scale = 1.0 / np.sqrt(DH)
    bf = ml_dtypes.bfloat16

    def slab(a):
        # [rows, cols] -> [128, ktile, cols]
        return np.ascontiguousarray(
            a.reshape(a.shape[0] // P, P, a.shape[1]).transpose(1, 0, 2)
        )

    in_maps = []
    for c in range(N_CORES):
        b, hf = divmod(c, 2)
        cols = slice(hf * DQ, (hf + 1) * DQ)
        in_maps.append(
            {
                "xt": slab(x[b].T.astype(bf)),
                "yt": slab(y[b].T.astype(bf)),
                "wq": slab((Wq[:, cols] * scale).astype(bf)),
                "wk": slab(Wk[:, cols].astype(bf)),
                "wv": slab(Wv[:, cols].astype(bf)),
                "wo": slab(Wo[cols, :].astype(bf)),
                "biasd": np.ascontiguousarray(bias[b, 0, 0].reshape(NK, P).T),
                "onesd": np.ones((P, NHL), dtype=bf),
            }
        )
    return in_maps


def get_program():
    global _CACHED_NC
    if _CACHED_NC is None:
        _CACHED_NC = build_program()
    return _CACHED_NC


def kernel(x, y, bias, Wq, Wk, Wv, Wo):
    nc = get_program()
    in_maps = _prep_in_maps(x, y, bias, Wq, Wk, Wv, Wo)
    res = bass_utils.run_bass_kernel_spmd(nc, in_maps, core_ids=list(range(N_CORES)))
    B = 4
    out = np.empty((B, S, HID), dtype=np.float32)
    for b in range(B):
        out[b] = res.results[2 * b]["out"].astype(np.float32) + res.results[
            2 * b + 1
        ]["out"].astype(np.float32)
    return out


# revision 47
# speedup vs baseline: 14.7325x; 14.7325x over previous
"""Multi-head attention (B=4, S=1024, H=1024, 16 heads) on 8 trn2 cores.

Sharding: 8 shards = (batch b in 0..3) x (head-half hf in 0..1).
Each core computes attention for 8 heads of one batch and a partial
output projection (row-parallel Wo); host sums the two partials per batch.

Per-core pipeline (matmuls bf16, PSUM fp32, output fp32):
  - inputs as [128, ktile, *] slabs so one or two DMAs load each tensor
    (HWDGE descriptor time, ~630ns each, would otherwise serialize startup)
  - QT/KT d-major bf16 slabs; V token-major bf16 with ones column per head
  - logitsT[k, q] per head via lhsT=KT tile (K=64), exp on ACT with
    per-partition bias fused (logits are O(+-9): fp32 exp, no max-sub)
  - attn@V reoriented: lhsT = exp tile [k, qtile], rhs = V_aug [k, 65]
    -> psum [q, 65] accumulated over k-tiles; col 64 = softmax denominator.
    Halves attn@V PE cycles vs moving the q dimension.
  - normalize: DVE reciprocal + per-partition scalar mul into [q, 128]
    head-pair tiles; DMA-XBAR transpose (free, idle DMA engines) back to
    [dq, tok] at-slabs for the output projection
  - Wo tail: each [tok-tile, hid-half] chunk accumulates all 4 dq-pairs in
    one PSUM group; psum->SBUF copies alternate DVE/ACT (ACT idle at tail);
    DMA per chunk overlaps the remaining matmuls
  - software pipelined: head h's attn@V rides inside head h+1's logits/exp
    stream; V bursts ride inside head 0's slots (wv arrives last)
"""

import numpy as np
import ml_dtypes

import concourse.bass as bass
import concourse.tile as tile
from concourse import bacc, mybir
from concourse import bass_utils

F32 = mybir.dt.float32
BF16 = mybir.dt.bfloat16
EXP = mybir.ActivationFunctionType.Exp

S = 1024  # sequence length (tokens)
HID = 1024  # model hidden
DQ = 512  # per-core projected dim (8 heads x 64)
NHL = 8  # local heads per core
DH = 64  # head depth
NK = HID // 128  # 8 contraction tiles over hidden
P = 128
N_CORES = 8

MM_DT = BF16

_CACHED_NC = None


def build_program(unroll=1):
    nc = bacc.Bacc("TRN2", target_bir_lowering=False, debug=False)
    # slab layouts [128, ktile, *]: host pre-rearranges; 1-2 DMAs per tensor
    xt = nc.dram_tensor("xt", [P, NK, S], BF16, kind="ExternalInput").ap()
    yt = nc.dram_tensor("yt", [P, NK, S], BF16, kind="ExternalInput").ap()
    wq = nc.dram_tensor("wq", [P, NK, DQ], BF16, kind="ExternalInput").ap()
    wk = nc.dram_tensor("wk", [P, NK, DQ], BF16, kind="ExternalInput").ap()
    wv = nc.dram_tensor("wv", [P, NK, DQ], BF16, kind="ExternalInput").ap()
    wo = nc.dram_tensor("wo", [P, 4, HID], BF16, kind="ExternalInput").ap()
    biasd = nc.dram_tensor("biasd", [P, NK], F32, kind="ExternalInput").ap()
    onesd = nc.dram_tensor("onesd", [P, NHL], BF16, kind="ExternalInput").ap()
    # bf16 partials: host upcasts and sums the two head-half partials
    out = nc.dram_tensor("out", [S, HID], BF16, kind="ExternalOutput").ap()

    with tile.TileContext(nc) as tc:
        for _ in range(unroll):
            emit_kernel(tc, out, xt, yt, wq, wk, wv, wo, biasd, onesd)
    nc.compile()
    return nc


def emit_kernel(tc, out, xt, yt, wq, wk, wv, wo, biasd, onesd):
    nc = tc.nc
    with (
        tc.tile_pool(name="wpool", bufs=1) as wpool,
        tc.tile_pool(name="qkv", bufs=1) as qkvpool,
        tc.tile_pool(name="atp", bufs=1) as atpool,
        tc.tile_pool(name="xypool", bufs=1) as xypool,
    ):
        # ---- batched slab DMA, critical-path first (exp stream is gated on
        # the Q then K projections): xt+wq, yt+wk, bias, then wv, wo.
        xt_sl = xypool.tile([P, NK, S], MM_DT, tag="xt", name="xt_sl")
        yt_sl = xypool.tile([P, NK, S], MM_DT, tag="yt", name="yt_sl")
        wq_sl = wpool.tile([P, NK, DQ], MM_DT, tag="wq", name="wq_sl")
        wk_sl = wpool.tile([P, NK, DQ], MM_DT, tag="wk", name="wk_sl")
        wv_sl = wpool.tile([P, NK, DQ], MM_DT, tag="wv", name="wv_sl")
        wo_sl = wpool.tile([P, 4, HID], MM_DT, tag="wo", name="wo_sl")
        # Input DMAs in strict priority order (xt+wq gate the Q projection,
        # yt+wk gate K and the first exp; wv/wo are needed late). First
        # chunks are single k-tiles so the first matmul starts sooner.
        for a, b in ((0, 1), (1, 4), (4, 8)):
            nc.sync.dma_start(xt_sl[:, a:b, :], xt[:, a:b, :])
            nc.scalar.dma_start(wq_sl[:, a:b, :], wq[:, a:b, :])
        for a, b in ((0, 4), (4, 8)):
            nc.sync.dma_start(yt_sl[:, a:b, :], yt[:, a:b, :])
            nc.scalar.dma_start(wk_sl[:, a:b, :], wk[:, a:b, :])
        bias_sb = wpool.tile([P, NK], F32, tag="bias")
        nc.scalar.dma_start(bias_sb[:], biasd[:])
        vones_sb = wpool.tile([P, NHL], BF16, tag="vones")
        nc.scalar.dma_start(vones_sb[:], onesd[:])
        nc.sync.dma_start(wv_sl[:, 0:4, :], wv[:, 0:4, :])
        nc.sync.dma_start(wv_sl[:, 4:8, :], wv[:, 4:8, :])
        nc.scalar.dma_start(wo_sl[:], wo[:])

        # ---- persistent slabs ----
        qt_sb = [qkvpool.tile([P, S], MM_DT, tag=f"qt{m}", name=f"qt{m}") for m in range(4)]
        kt_sb = [qkvpool.tile([P, S], MM_DT, tag=f"kt{m}", name=f"kt{m}") for m in range(4)]
        v_sb = [qkvpool.tile([P, NHL * (DH + 1)], MM_DT, tag=f"v{m}", name=f"v{m}") for m in range(8)]
        at_sb = [atpool.tile([P, S], MM_DT, tag=f"at{m}", name=f"at{m}") for m in range(4)]

        # PSUM (8 banks). Prologue: pro 4x[128,512]. Head loop:
        # lg 2x[128,1024]=4, av 2x[128,65]=2, pj 2x[128,512]=2.
        # Tail (all released): wo 6x[128,512].
        # PE p-state warm-up: ~3us of dummy matmuls on a memset tile while
        # the first input DMAs stream, so real matmuls run at full clock
        pp_warm = tc.alloc_tile_pool(name="pp_warm", bufs=2, space="PSUM")
        warm_sb = wpool.tile([P, 512], MM_DT, tag="warm", name="warm")
        nc.vector.memset(warm_sb[:], 0.0)
        for w in range(6):
            wps = pp_warm.tile([P, 512], F32, tag="warm", name="warmps")
            nc.tensor.matmul(wps[:], warm_sb[:, 0:P], warm_sb[:], start=True, stop=True)
        pp_warm.release()
        pp_pro = tc.alloc_tile_pool(name="pp_pro", bufs=4, space="PSUM")

        with (
            tc.tile_pool(name="expp", bufs=40) as exppool,
            tc.tile_pool(name="smallp", bufs=6) as smallpool,
            tc.tile_pool(name="aqp", bufs=2) as aqpool,
            tc.tile_pool(name="stagep", bufs=1) as stagepool,
        ):
            e_tiles = [[None] * NK for _ in range(NHL)]  # e[h][sk]
            aq_tiles = [None] * NK  # per qtile, current pair's [q,128] tile
            stage_sb = [
                stagepool.tile([P, 512], BF16, tag=f"st{g}", name=f"st{g}")
                for g in range(16)
            ]

            # ---- emission helpers ----
            def emit_v_burst(m, pool):
                # V projection for token tile m (token-major, ones appended)
                ps = pool.tile([P, DQ], F32, tag="pj", name="pjv")
                for k in range(NK):
                    nc.tensor.matmul(
                        ps[:],
                        yt_sl[:, k, m * P : (m + 1) * P],
                        wv_sl[:, k, :],
                        start=(k == 0),
                        stop=(k == NK - 1),
                    )
                dst3 = v_sb[m][:].rearrange("p (h c) -> p h c", c=DH + 1)
                src3 = ps[:].rearrange("p (h c) -> p h c", c=DH)
                nc.vector.tensor_copy(dst3[:, :, 0:DH], src3[:, :, :])
                nc.vector.tensor_copy(
                    dst3[:, :, DH : DH + 1],
                    vones_sb[:].rearrange("p (a b) -> p a b", b=1),
                )

            def emit_proj_burst(pair, which, n, pool):
                # one [128dq, 512tok] chunk of the q/k projection for `pair`
                w_sl, src_sl, dst = (
                    (wq_sl, xt_sl, qt_sb) if which == "q" else (wk_sl, yt_sl, kt_sb)
                )
                ps = pool.tile([P, 512], F32, tag="pj", name="pj")
                for k in range(NK):
                    nc.tensor.matmul(
                        ps[:],
                        w_sl[:, k, pair * P : (pair + 1) * P],
                        src_sl[:, k, n * 512 : (n + 1) * 512],
                        start=(k == 0),
                        stop=(k == NK - 1),
                    )
                nc.vector.tensor_copy(dst[pair][:, n * 512 : (n + 1) * 512], ps[:])

            def emit_lg_exp(h, sk):
                pair, hi = divmod(h, 2)
                base = hi * DH
                lg = pp_lg.tile([P, S], F32, tag="lg", name="lg")
                for n in range(2):
                    nc.tensor.matmul(
                        lg[:, n * 512 : (n + 1) * 512],
                        kt_sb[pair][base : base + DH, sk * P : (sk + 1) * P],
                        qt_sb[pair][base : base + DH, n * 512 : (n + 1) * 512],
                        start=True,
                        stop=True,
                    )
                e = exppool.tile([P, S], MM_DT, tag="exp", name="exp")
                nc.scalar.activation(e[:], lg[:], EXP, bias=bias_sb[:, sk : sk + 1])
                return e

            def emit_av(h, qtile, pool=None):
                # attn@V for (head h, query tile qtile): psum [128q, 65]
                pair, hi = divmod(h, 2)
                av = (pool or pp_av).tile([P, DH + 1], F32, tag="av", name="av")
                for sk in range(NK):
                    nc.tensor.matmul(
                        av[:],
                        e_tiles[h][sk][:, qtile * P : (qtile + 1) * P],
                        v_sb[sk][:, h * (DH + 1) : (h + 1) * (DH + 1)],
                        start=(sk == 0),
                        stop=(sk == NK - 1),
                    )
                rc = smallpool.tile([P, 1], F32, tag="rc", name="rc")
                nc.vector.reciprocal(rc[:], av[:, DH : DH + 1])
                if hi == 0:
                    aq = aqpool.tile([P, P], MM_DT, tag=f"aq{qtile}", name=f"aq{qtile}")
                    aq_tiles[qtile] = aq
                else:
                    aq = aq_tiles[qtile]
                nc.vector.tensor_scalar_mul(
                    aq[:, hi * DH : (hi + 1) * DH], av[:, 0:DH], rc[:]
                )
                if hi == 1:
                    nc.sync.dma_start_transpose(
                        at_sb[pair][:, qtile * P : (qtile + 1) * P], aq[:]
                    )

            # ---- prologue: Q projections for ALL pairs (xt/wq arrive
            # first; PE otherwise idles in the DMA-paced window), then K for
            # pair 0 (the gate for the first logits/exp). Full-row N=1024
            # matmuls: one weight load per k-tile.
            for pr in range(4):
                for n in range(2):
                    emit_proj_burst(pr, "q", n, pp_pro)
            for n in range(2):
                emit_proj_burst(0, "k", n, pp_pro)
            pp_pro.release()
            pp_av = tc.alloc_tile_pool(name="pp_av", bufs=2, space="PSUM")
            pp_pj = tc.alloc_tile_pool(name="pp_pj", bufs=2, space="PSUM")
            pp_lg = tc.alloc_tile_pool(name="pp_lg", bufs=2, space="PSUM")

            # ---- software-pipelined head loop ----
            # Per block h: lg/exp stream (the ACT pace-setter) + fillers
            # (remaining K projections, V bursts once wv lands, lagged attn@V
            # groups), balanced so no block's PE work exceeds the ACT pace.
            fillers = {
                (0, 2): lambda: emit_proj_burst(1, "k", 0, pp_pj),
                (0, 5): lambda: emit_proj_burst(1, "k", 1, pp_pj),
                (1, 1): lambda: emit_v_burst(0, pp_pj),
                (1, 3): lambda: emit_v_burst(1, pp_pj),
                (1, 5): lambda: emit_v_burst(2, pp_pj),
                (2, 1): lambda: emit_v_burst(3, pp_pj),
                (2, 4): lambda: emit_v_burst(4, pp_pj),
                (2, 6): lambda: emit_proj_burst(2, "k", 0, pp_pj),
                (3, 1): lambda: emit_v_burst(5, pp_pj),
                (3, 3): lambda: emit_v_burst(6, pp_pj),
                (3, 5): lambda: emit_v_burst(7, pp_pj),
                (3, 7): lambda: emit_proj_burst(2, "k", 1, pp_pj),
                (4, 2): lambda: emit_proj_burst(3, "k", 0, pp_pj),
                (5, 2): lambda: emit_proj_burst(3, "k", 1, pp_pj),
            }

            av_sched = {4: [0], 5: [1], 6: [2, 3], 7: [4, 5]}
            for h in range(NHL):
                for sk in range(NK):
                    e_tiles[h][sk] = emit_lg_exp(h, sk)
                    f = fillers.get((h, sk))
                    if f is not None:
                        f()
                    for avh in av_sched.get(h, ()):
                        emit_av(avh, sk)
            for qtile in range(NK):
                emit_av(6, qtile)
            # ---- tail: av(7) drain on a deeper PSUM pool (3 groups in
            # flight), then Wo: 4-pair PSUM accumulation per output chunk,
            # psum->SBUF copies alternating DVE/ACT, DMA per chunk
            pp_lg.release()
            pp_pj.release()
            pp_av2 = tc.alloc_tile_pool(name="pp_av2", bufs=2, space="PSUM")
            pp_wo = tc.alloc_tile_pool(name="pp_wo", bufs=3, space="PSUM")

            def emit_wo(m, n):
                g = 2 * m + n
                ps = pp_wo.tile([P, 512], F32, tag="wops", name="wops")
                for pair in range(4):
                    nc.tensor.matmul(
                        ps[:],
                        at_sb[pair][:, m * P : (m + 1) * P],
                        wo_sl[:, pair : pair + 1, n * 512 : (n + 1) * 512],
                        start=(pair == 0),
                        stop=(pair == 3),
                    )
                if g % 2 == 0:
                    nc.vector.tensor_copy(stage_sb[g][:], ps[:])
                else:
                    nc.scalar.copy(stage_sb[g][:], ps[:])
                (nc.sync if g % 2 == 0 else nc.scalar).dma_start(
                    out[m * P : (m + 1) * P, n * 512 : (n + 1) * 512], stage_sb[g][:]
                )

            for qtile in range(NK):
                emit_av(NHL - 1, qtile,
                        pool=(pp_av2 if qtile % 2 == 0 else pp_av))
            for qtile in range(NK):
                emit_wo(qtile, 0)
                emit_wo(qtile, 1)
            pp_wo.release()
            pp_av2.release()
            pp_av.release()


def _prep_in_maps(x, y, bias, Wq, Wk, Wv, Wo):
    x = np.asarray(x, dtype=np.float32)
    y = np.asarray(y, dtype=np.float32)
    bias = np.asarray(bias, dtype=np.float32)
    Wq = np.asarray(Wq, dtype=np.float32)
    Wk = np.asarray(Wk, dtype=np.float32)
    Wv = np.asarray(Wv, dtype=np.float32)
    Wo = np.asarray(Wo, dtype=np.float32)
    scale = 1.0 / np.sqrt(DH)
    bf = ml_dtypes.bfloat16

    def slab(a):
        # [rows, cols] -> [128, ktile, cols]
        return np.ascontiguousarray(
            a.reshape(a.shape[0] // P, P, a.shape[1]).transpose(1, 0, 2)
        )

    in_maps = []
    for c in range(N_CORES):
        b, hf = divmod(c, 2)
        cols = slice(hf * DQ, (hf + 1) * DQ)
        in_maps.append(
            {
                "xt": slab(x[b].T.astype(bf)),
                "yt": slab(y[b].T.astype(bf)),
                "wq": slab((Wq[:, cols] * scale).astype(bf)),
                "wk": slab(Wk[:, cols].astype(bf)),
                "wv": slab(Wv[:, cols].astype(bf)),
                "wo": slab(Wo[cols, :].astype(bf)),
                "biasd": np.ascontiguousarray(bias[b, 0, 0].reshape(NK, P).T),
                "onesd": np.ones((P, NHL), dtype=bf),
            }
        )
    return in_maps


def get_program():
    global _CACHED_NC
    if _CACHED_NC is None:
        _CACHED_NC = build_program()
    return _CACHED_NC


def kernel(x, y, bias, Wq, Wk, Wv, Wo):
    nc = get_program()
    in_maps = _prep_in_maps(x, y, bias, Wq, Wk, Wv, Wo)
    res = bass_utils.run_bass_kernel_spmd(nc, in_maps, core_ids=list(range(N_CORES)))
    B = 4
    out = np.empty((B, S, HID), dtype=np.float32)
    for b in range(B):
        out[b] = res.results[2 * b]["out"].astype(np.float32) + res.results[
            2 * b + 1
        ]["out"].astype(np.float32)
    return out


# revision 49
# speedup vs baseline: 15.9852x; 1.0850x over previous
"""Multi-head attention (B=4, S=1024, H=1024, 16 heads) on 8 trn2 cores.

Sharding: 8 shards = (batch b in 0..3) x (head-half hf in 0..1).
Each core computes attention for 8 heads of one batch and a partial
output projection (row-parallel Wo); host sums the two partials per batch.

Per-core pipeline (matmuls bf16, PSUM fp32, output fp32):
  - inputs as [128, ktile, *] slabs so one or two DMAs load each tensor
    (HWDGE descriptor time, ~630ns each, would otherwise serialize startup)
  - QT/KT d-major bf16 slabs; V token-major bf16 with ones column per head
  - logitsT[k, q] per head via lhsT=KT tile (K=64), exp on ACT with
    per-partition bias fused (logits are O(+-9): fp32 exp, no max-sub)
  - attn@V reoriented: lhsT = exp tile [k, qtile], rhs = V_aug [k, 65]
    -> psum [q, 65] accumulated over k-tiles; col 64 = softmax denominator.
    Halves attn@V PE cycles vs moving the q dimension.
  - normalize: DVE reciprocal + per-partition scalar mul into [q, 128]
    head-pair tiles; DMA-XBAR transpose (free, idle DMA engines) back to
    [dq, tok] at-slabs for the output projection
  - Wo tail: each [tok-tile, hid-half] chunk accumulates all 4 dq-pairs in
    one PSUM group; psum->SBUF copies alternate DVE/ACT (ACT idle at tail);
    DMA per chunk overlaps the remaining matmuls
  - software pipelined: head h's attn@V rides inside head h+1's logits/exp
    stream; V bursts ride inside head 0's slots (wv arrives last)
"""

import numpy as np
import ml_dtypes

import concourse.bass as bass
import concourse.tile as tile
from concourse import bacc, mybir
from concourse import bass_utils

F32 = mybir.dt.float32
BF16 = mybir.dt.bfloat16
EXP = mybir.ActivationFunctionType.Exp

S = 1024  # sequence length (tokens)
HID = 1024  # model hidden
DQ = 512  # per-core projected dim (8 heads x 64)
NHL = 8  # local heads per core
DH = 64  # head depth
NK = HID // 128  # 8 contraction tiles over hidden
P = 128
N_CORES = 8

MM_DT = BF16

_CACHED_NC = None


def build_program(unroll=1):
    nc = bacc.Bacc("TRN2", target_bir_lowering=False, debug=False)
    # slab layouts [128, ktile, *]: host pre-rearranges; 1-2 DMAs per tensor
    xt = nc.dram_tensor("xt", [P, NK, S], BF16, kind="ExternalInput").ap()
    yt = nc.dram_tensor("yt", [P, NK, S], BF16, kind="ExternalInput").ap()
    wq = nc.dram_tensor("wq", [P, NK, DQ], BF16, kind="ExternalInput").ap()
    wk = nc.dram_tensor("wk", [P, NK, DQ], BF16, kind="ExternalInput").ap()
    wv = nc.dram_tensor("wv", [P, NK, DQ], BF16, kind="ExternalInput").ap()
    wo = nc.dram_tensor("wo", [P, 4, HID], BF16, kind="ExternalInput").ap()
    biasd = nc.dram_tensor("biasd", [P, NK], F32, kind="ExternalInput").ap()
    onesd = nc.dram_tensor("onesd", [P, NHL], BF16, kind="ExternalInput").ap()
    # bf16 partials: host upcasts and sums the two head-half partials
    out = nc.dram_tensor("out", [S, HID], BF16, kind="ExternalOutput").ap()

    with tile.TileContext(nc) as tc:
        for _ in range(unroll):
            emit_kernel(tc, out, xt, yt, wq, wk, wv, wo, biasd, onesd)
    nc.compile()
    return nc


def emit_kernel(tc, out, xt, yt, wq, wk, wv, wo, biasd, onesd):
    nc = tc.nc
    with (
        tc.tile_pool(name="wpool", bufs=1) as wpool,
        tc.tile_pool(name="qkv", bufs=1) as qkvpool,
        tc.tile_pool(name="atp", bufs=1) as atpool,
        tc.tile_pool(name="xypool", bufs=1) as xypool,
    ):
        # ---- batched slab DMA, critical-path first (exp stream is gated on
        # the Q then K projections): xt+wq, yt+wk, bias, then wv, wo.
        xt_sl = xypool.tile([P, NK, S], MM_DT, tag="xt", name="xt_sl")
        yt_sl = xypool.tile([P, NK, S], MM_DT, tag="yt", name="yt_sl")
        wq_sl = wpool.tile([P, NK, DQ], MM_DT, tag="wq", name="wq_sl")
        wk_sl = wpool.tile([P, NK, DQ], MM_DT, tag="wk", name="wk_sl")
        wv_sl = wpool.tile([P, NK, DQ], MM_DT, tag="wv", name="wv_sl")
        wo_sl = wpool.tile([P, 4, HID], MM_DT, tag="wo", name="wo_sl")
        # Input DMAs in strict priority order (xt+wq gate the Q projection,
        # yt+wk gate K and the first exp; wv/wo are needed late). First
        # chunks are single k-tiles so the first matmul starts sooner.
        for a, b in ((0, 1), (1, 4), (4, 8)):
            nc.sync.dma_start(xt_sl[:, a:b, :], xt[:, a:b, :])
            nc.scalar.dma_start(wq_sl[:, a:b, :], wq[:, a:b, :])
        for a, b in ((0, 4), (4, 8)):
            nc.sync.dma_start(yt_sl[:, a:b, :], yt[:, a:b, :])
            nc.scalar.dma_start(wk_sl[:, a:b, :], wk[:, a:b, :])
        bias_sb = wpool.tile([P, NK], F32, tag="bias")
        nc.scalar.dma_start(bias_sb[:], biasd[:])
        vones_sb = wpool.tile([P, NHL], BF16, tag="vones")
        nc.scalar.dma_start(vones_sb[:], onesd[:])
        nc.sync.dma_start(wv_sl[:, 0:4, :], wv[:, 0:4, :])
        nc.sync.dma_start(wv_sl[:, 4:8, :], wv[:, 4:8, :])
        nc.scalar.dma_start(wo_sl[:], wo[:])

        # ---- persistent slabs ----
        qt_sb = [qkvpool.tile([P, S], MM_DT, tag=f"qt{m}", name=f"qt{m}") for m in range(4)]
        kt_sb = [qkvpool.tile([P, S], MM_DT, tag=f"kt{m}", name=f"kt{m}") for m in range(4)]
        v_sb = [qkvpool.tile([P, NHL * (DH + 1)], MM_DT, tag=f"v{m}", name=f"v{m}") for m in range(8)]
        at_sb = [atpool.tile([P, S], MM_DT, tag=f"at{m}", name=f"at{m}") for m in range(4)]

        # PSUM (8 banks). Prologue: pro 4x[128,512]. Head loop:
        # lg 2x[128,1024]=4, av 2x[128,65]=2, pj 2x[128,512]=2.
        # Tail (all released): wo 6x[128,512].
        # PE p-state warm-up: ~3us of dummy matmuls on a memset tile while
        # the first input DMAs stream, so real matmuls run at full clock
        pp_warm = tc.alloc_tile_pool(name="pp_warm", bufs=2, space="PSUM")
        warm_sb = wpool.tile([P, 512], MM_DT, tag="warm", name="warm")
        nc.vector.memset(warm_sb[:], 0.0)
        for w in range(6):
            wps = pp_warm.tile([P, 512], F32, tag="warm", name="warmps")
            nc.tensor.matmul(wps[:], warm_sb[:, 0:P], warm_sb[:], start=True, stop=True)
        pp_warm.release()
        pp_pro = tc.alloc_tile_pool(name="pp_pro", bufs=4, space="PSUM")

        with (
            tc.tile_pool(name="expp", bufs=40) as exppool,
            tc.tile_pool(name="smallp", bufs=6) as smallpool,
            tc.tile_pool(name="aqp", bufs=2) as aqpool,
            tc.tile_pool(name="stagep", bufs=1) as stagepool,
        ):
            e_tiles = [[None] * NK for _ in range(NHL)]  # e[h][sk]
            aq_tiles = [None] * NK  # per qtile, current pair's [q,128] tile
            stage_sb = [
                stagepool.tile([P, 512], BF16, tag=f"st{g}", name=f"st{g}")
                for g in range(16)
            ]

            # ---- emission helpers ----
            def emit_v_burst(m, pool):
                # V projection for token tile m (token-major, ones appended)
                ps = pool.tile([P, DQ], F32, tag="pj", name="pjv")
                for k in range(NK):
                    nc.tensor.matmul(
                        ps[:],
                        yt_sl[:, k, m * P : (m + 1) * P],
                        wv_sl[:, k, :],
                        start=(k == 0),
                        stop=(k == NK - 1),
                    )
                dst3 = v_sb[m][:].rearrange("p (h c) -> p h c", c=DH + 1)
                src3 = ps[:].rearrange("p (h c) -> p h c", c=DH)
                nc.vector.tensor_copy(dst3[:, :, 0:DH], src3[:, :, :])
                nc.vector.tensor_copy(
                    dst3[:, :, DH : DH + 1],
                    vones_sb[:].rearrange("p (a b) -> p a b", b=1),
                )

            def emit_proj_burst(pair, which, n, pool):
                # one [128dq, 512tok] chunk of the q/k projection for `pair`
                w_sl, src_sl, dst = (
                    (wq_sl, xt_sl, qt_sb) if which == "q" else (wk_sl, yt_sl, kt_sb)
                )
                ps = pool.tile([P, 512], F32, tag="pj", name="pj")
                for k in range(NK):
                    nc.tensor.matmul(
                        ps[:],
                        w_sl[:, k, pair * P : (pair + 1) * P],
                        src_sl[:, k, n * 512 : (n + 1) * 512],
                        start=(k == 0),
                        stop=(k == NK - 1),
                    )
                nc.vector.tensor_copy(dst[pair][:, n * 512 : (n + 1) * 512], ps[:])

            def emit_lg_exp(h, sk):
                pair, hi = divmod(h, 2)
                base = hi * DH
                lg = pp_lg.tile([P, S], F32, tag="lg", name="lg")
                for n in range(2):
                    nc.tensor.matmul(
                        lg[:, n * 512 : (n + 1) * 512],
                        kt_sb[pair][base : base + DH, sk * P : (sk + 1) * P],
                        qt_sb[pair][base : base + DH, n * 512 : (n + 1) * 512],
                        start=True,
                        stop=True,
                    )
                e = exppool.tile([P, S], MM_DT, tag="exp", name="exp")
                nc.scalar.activation(e[:], lg[:], EXP, bias=bias_sb[:, sk : sk + 1])
                return e

            def emit_av(h, qtile, pool=None):
                # attn@V for (head h, query tile qtile): psum [128q, 65]
                pair, hi = divmod(h, 2)
                av = (pool or pp_av).tile([P, DH + 1], F32, tag="av", name="av")
                for sk in range(NK):
                    nc.tensor.matmul(
                        av[:],
                        e_tiles[h][sk][:, qtile * P : (qtile + 1) * P],
                        v_sb[sk][:, h * (DH + 1) : (h + 1) * (DH + 1)],
                        start=(sk == 0),
                        stop=(sk == NK - 1),
                    )
                rc = smallpool.tile([P, 1], F32, tag="rc", name="rc")
                nc.vector.reciprocal(rc[:], av[:, DH : DH + 1])
                if hi == 0:
                    aq = aqpool.tile([P, P], MM_DT, tag=f"aq{qtile}", name=f"aq{qtile}")
                    aq_tiles[qtile] = aq
                else:
                    aq = aq_tiles[qtile]
                nc.vector.tensor_scalar_mul(
                    aq[:, hi * DH : (hi + 1) * DH], av[:, 0:DH], rc[:]
                )
                if hi == 1:
                    nc.sync.dma_start_transpose(
                        at_sb[pair][:, qtile * P : (qtile + 1) * P], aq[:]
                    )

            # ---- prologue: Q projections for ALL pairs (xt/wq arrive
            # first; PE otherwise idles in the DMA-paced window), then K for
            # pair 0 (the gate for the first logits/exp). Full-row N=1024
            # matmuls: one weight load per k-tile.
            for pr in range(4):
                for n in range(2):
                    emit_proj_burst(pr, "q", n, pp_pro)
            for n in range(2):
                emit_proj_burst(0, "k", n, pp_pro)
            pp_pro.release()
            pp_av = tc.alloc_tile_pool(name="pp_av", bufs=2, space="PSUM")
            pp_pj = tc.alloc_tile_pool(name="pp_pj", bufs=2, space="PSUM")
            pp_lg = tc.alloc_tile_pool(name="pp_lg", bufs=2, space="PSUM")

            # ---- software-pipelined head loop ----
            # Per block h: lg/exp stream (the ACT pace-setter) + fillers
            # (remaining K projections, V bursts once wv lands, lagged attn@V
            # groups), balanced so no block's PE work exceeds the ACT pace.
            fillers = {
                (0, 2): lambda: emit_proj_burst(1, "k", 0, pp_pj),
                (0, 5): lambda: emit_proj_burst(1, "k", 1, pp_pj),
                (1, 1): lambda: emit_v_burst(0, pp_pj),
                (1, 3): lambda: emit_v_burst(1, pp_pj),
                (1, 5): lambda: emit_v_burst(2, pp_pj),
                (2, 1): lambda: emit_v_burst(3, pp_pj),
                (2, 4): lambda: emit_v_burst(4, pp_pj),
                (2, 6): lambda: emit_proj_burst(2, "k", 0, pp_pj),
                (3, 1): lambda: emit_v_burst(5, pp_pj),
                (3, 3): lambda: emit_v_burst(6, pp_pj),
                (3, 5): lambda: emit_v_burst(7, pp_pj),
                (3, 7): lambda: emit_proj_burst(2, "k", 1, pp_pj),
                (4, 2): lambda: emit_proj_burst(3, "k", 0, pp_pj),
                (5, 2): lambda: emit_proj_burst(3, "k", 1, pp_pj),
            }

            av_sched = {4: [0], 5: [1], 6: [2, 3], 7: [4, 5]}
            for h in range(NHL):
                for sk in range(NK):
                    e_tiles[h][sk] = emit_lg_exp(h, sk)
                    f = fillers.get((h, sk))
                    if f is not None:
                        f()
                    for avh in av_sched.get(h, ()):
                        emit_av(avh, sk)
            for qtile in range(NK):
                emit_av(6, qtile)
            # ---- tail: av(7) drain on a deeper PSUM pool (3 groups in
            # flight), then Wo: 4-pair PSUM accumulation per output chunk,
            # psum->SBUF copies alternating DVE/ACT, DMA per chunk
            pp_lg.release()
            pp_pj.release()
            pp_av2 = tc.alloc_tile_pool(name="pp_av2", bufs=2, space="PSUM")
            pp_wo = tc.alloc_tile_pool(name="pp_wo", bufs=3, space="PSUM")

            def emit_wo(m, n):
                g = 2 * m + n
                ps = pp_wo.tile([P, 512], F32, tag="wops", name="wops")
                for pair in range(4):
                    nc.tensor.matmul(
                        ps[:],
                        at_sb[pair][:, m * P : (m + 1) * P],
                        wo_sl[:, pair : pair + 1, n * 512 : (n + 1) * 512],
                        start=(pair == 0),
                        stop=(pair == 3),
                    )
                if g % 2 == 0:
                    nc.vector.tensor_copy(stage_sb[g][:], ps[:])
                else:
                    nc.scalar.copy(stage_sb[g][:], ps[:])
                (nc.sync if g % 2 == 0 else nc.scalar).dma_start(
                    out[m * P : (m + 1) * P, n * 512 : (n + 1) * 512], stage_sb[g][:]
                )

            for qtile in range(NK):
                emit_av(NHL - 1, qtile,
                        pool=(pp_av2 if qtile % 2 == 0 else pp_av))
            for qtile in range(NK):
                emit_wo(qtile, 0)
                emit_wo(qtile, 1)
            pp_wo.release()
            pp_av2.release()
            pp_av.release()


def _prep_in_maps(x, y, bias, Wq, Wk, Wv, Wo):
    x = np.asarray(x, dtype=np.float32)
    y = np.asarray(y, dtype=np.float32)
    bias = np.asarray(bias, dtype=np.float32)
    Wq = np.asarray(Wq, dtype=np.float32)
    Wk = np.asarray(Wk, dtype=np.float32)
    Wv = np.asarray(Wv, dtype=np.float32)
    Wo = np.asarray(Wo, dtype=np.float32)
    scale = 1.0 / np.sqrt(DH)
    bf = ml_dtypes.bfloat16

    def slab(a):
        # [rows, cols] -> [128, ktile, cols]
        return np.ascontiguousarray(
            a.reshape(a.shape[0] // P, P, a.shape[1]).transpose(1, 0, 2)
        )

    in_maps = []
    for c in range(N_CORES):
        b, hf = divmod(c, 2)
        cols = slice(hf * DQ, (hf + 1) * DQ)
        in_maps.append(
            {
                "xt": slab(x[b].T.astype(bf)),
                "yt": slab(y[b].T.astype(bf)),
                "wq": slab((Wq[:, cols] * scale).astype(bf)),
                "wk": slab(Wk[:, cols].astype(bf)),
                "wv": slab(Wv[:, cols].astype(bf)),
                "wo": slab(Wo[cols, :].astype(bf)),
                "biasd": np.ascontiguousarray(bias[b, 0, 0].reshape(NK, P).T),
                "onesd": np.ones((P, NHL), dtype=bf),
            }
        )
    return in_maps


def get_program():
    global _CACHED_NC
    if _CACHED_NC is None:
        _CACHED_NC = build_program()
    return _CACHED_NC


def kernel(x, y, bias, Wq, Wk, Wv, Wo):
    nc = get_program()
    in_maps = _prep_in_maps(x, y, bias, Wq, Wk, Wv, Wo)
    res = bass_utils.run_bass_kernel_spmd(nc, in_maps, core_ids=list(range(N_CORES)))
    B = 4
    out = np.empty((B, S, HID), dtype=np.float32)
    for b in range(B):
        out[b] = res.results[2 * b]["out"].astype(np.float32) + res.results[
            2 * b + 1
        ]["out"].astype(np.float32)
    return out


# revision 50
# speedup vs baseline: 15.9951x; 1.0006x over previous
"""Multi-head attention (B=4, S=1024, H=1024, 16 heads) on 8 trn2 cores.

Sharding: 8 shards = (batch b in 0..3) x (head-half hf in 0..1).
Each core computes attention for 8 heads of one batch and a partial
output projection (row-parallel Wo); host sums the two partials per batch.

Per-core pipeline (matmuls bf16, PSUM fp32, output fp32):
  - inputs as [128, ktile, *] slabs so one or two DMAs load each tensor
    (HWDGE descriptor time, ~630ns each, would otherwise serialize startup)
  - QT/KT d-major bf16 slabs; V token-major bf16 with ones column per head
  - logitsT[k, q] per head via lhsT=KT tile (K=64), exp on ACT with
    per-partition bias fused (logits are O(+-9): fp32 exp, no max-sub)
  - attn@V reoriented: lhsT = exp tile [k, qtile], rhs = V_aug [k, 65]
    -> psum [q, 65] accumulated over k-tiles; col 64 = softmax denominator.
    Halves attn@V PE cycles vs moving the q dimension.
  - normalize: DVE reciprocal + per-partition scalar mul into [q, 128]
    head-pair tiles; DMA-XBAR transpose (free, idle DMA engines) back to
    [dq, tok] at-slabs for the output projection
  - Wo tail: each [tok-tile, hid-half] chunk accumulates all 4 dq-pairs in
    one PSUM group; psum->SBUF copies alternate DVE/ACT (ACT idle at tail);
    DMA per chunk overlaps the remaining matmuls
  - software pipelined: head h's attn@V rides inside head h+1's logits/exp
    stream; V bursts ride inside head 0's slots (wv arrives last)
"""

import numpy as np
import ml_dtypes

import concourse.bass as bass
import concourse.tile as tile
from concourse import bacc, mybir
from concourse import bass_utils

F32 = mybir.dt.float32
BF16 = mybir.dt.bfloat16
EXP = mybir.ActivationFunctionType.Exp

S = 1024  # sequence length (tokens)
HID = 1024  # model hidden
DQ = 512  # per-core projected dim (8 heads x 64)
NHL = 8  # local heads per core
DH = 64  # head depth
NK = HID // 128  # 8 contraction tiles over hidden
P = 128
N_CORES = 8

MM_DT = BF16

_CACHED_NC = None


def build_program(unroll=1):
    nc = bacc.Bacc("TRN2", target_bir_lowering=False, debug=False)
    # slab layouts [128, ktile, *]: host pre-rearranges; 1-2 DMAs per tensor
    xt = nc.dram_tensor("xt", [P, NK, S], BF16, kind="ExternalInput").ap()
    yt = nc.dram_tensor("yt", [P, NK, S], BF16, kind="ExternalInput").ap()
    wq = nc.dram_tensor("wq", [P, NK, DQ], BF16, kind="ExternalInput").ap()
    wk = nc.dram_tensor("wk", [P, NK, DQ], BF16, kind="ExternalInput").ap()
    wv = nc.dram_tensor("wv", [P, NK, DQ], BF16, kind="ExternalInput").ap()
    wo = nc.dram_tensor("wo", [P, 4, HID], BF16, kind="ExternalInput").ap()
    biasd = nc.dram_tensor("biasd", [P, NK], F32, kind="ExternalInput").ap()
    onesd = nc.dram_tensor("onesd", [P, NHL], BF16, kind="ExternalInput").ap()
    # bf16 partials: host upcasts and sums the two head-half partials
    out = nc.dram_tensor("out", [S, HID], BF16, kind="ExternalOutput").ap()

    with tile.TileContext(nc) as tc:
        for _ in range(unroll):
            emit_kernel(tc, out, xt, yt, wq, wk, wv, wo, biasd, onesd)
    nc.compile()
    return nc


def emit_kernel(tc, out, xt, yt, wq, wk, wv, wo, biasd, onesd):
    nc = tc.nc
    with (
        tc.tile_pool(name="wpool", bufs=1) as wpool,
        tc.tile_pool(name="qkv", bufs=1) as qkvpool,
        tc.tile_pool(name="atp", bufs=1) as atpool,
        tc.tile_pool(name="xypool", bufs=1) as xypool,
    ):
        # ---- batched slab DMA, critical-path first (exp stream is gated on
        # the Q then K projections): xt+wq, yt+wk, bias, then wv, wo.
        xt_sl = xypool.tile([P, NK, S], MM_DT, tag="xt", name="xt_sl")
        yt_sl = xypool.tile([P, NK, S], MM_DT, tag="yt", name="yt_sl")
        wq_sl = wpool.tile([P, NK, DQ], MM_DT, tag="wq", name="wq_sl")
        wk_sl = wpool.tile([P, NK, DQ], MM_DT, tag="wk", name="wk_sl")
        wv_sl = wpool.tile([P, NK, DQ], MM_DT, tag="wv", name="wv_sl")
        wo_sl = wpool.tile([P, 4, HID], MM_DT, tag="wo", name="wo_sl")
        # Input DMAs in strict priority order (xt+wq gate the Q projection,
        # yt+wk gate K and the first exp; wv/wo are needed late). First
        # chunks are single k-tiles so the first matmul starts sooner.
        for a, b in ((0, 1), (1, 2), (2, 4), (4, 8)):
            nc.sync.dma_start(xt_sl[:, a:b, :], xt[:, a:b, :])
            nc.scalar.dma_start(wq_sl[:, a:b, :], wq[:, a:b, :])
        for a, b in ((0, 2), (2, 4), (4, 8)):
            nc.sync.dma_start(yt_sl[:, a:b, :], yt[:, a:b, :])
            nc.scalar.dma_start(wk_sl[:, a:b, :], wk[:, a:b, :])
        bias_sb = wpool.tile([P, NK], F32, tag="bias")
        nc.scalar.dma_start(bias_sb[:], biasd[:])
        vones_sb = wpool.tile([P, NHL], BF16, tag="vones")
        nc.scalar.dma_start(vones_sb[:], onesd[:])
        nc.sync.dma_start(wv_sl[:, 0:4, :], wv[:, 0:4, :])
        nc.sync.dma_start(wv_sl[:, 4:8, :], wv[:, 4:8, :])
        nc.scalar.dma_start(wo_sl[:], wo[:])

        # ---- persistent slabs ----
        qt_sb = [qkvpool.tile([P, S], MM_DT, tag=f"qt{m}", name=f"qt{m}") for m in range(4)]
        kt_sb = [qkvpool.tile([P, S], MM_DT, tag=f"kt{m}", name=f"kt{m}") for m in range(4)]
        v_sb = [qkvpool.tile([P, NHL * (DH + 1)], MM_DT, tag=f"v{m}", name=f"v{m}") for m in range(8)]
        at_sb = [atpool.tile([P, S], MM_DT, tag=f"at{m}", name=f"at{m}") for m in range(4)]

        # PSUM (8 banks). Prologue: pro 4x[128,512]. Head loop:
        # lg 2x[128,1024]=4, av 2x[128,65]=2, pj 2x[128,512]=2.
        # Tail (all released): wo 6x[128,512].
        # PE p-state warm-up: ~3us of dummy matmuls on a memset tile while
        # the first input DMAs stream, so real matmuls run at full clock
        pp_warm = tc.alloc_tile_pool(name="pp_warm", bufs=2, space="PSUM")
        warm_sb = wpool.tile([P, 512], MM_DT, tag="warm", name="warm")
        nc.vector.memset(warm_sb[:], 0.0)
        for w in range(6):
            wps = pp_warm.tile([P, 512], F32, tag="warm", name="warmps")
            nc.tensor.matmul(wps[:], warm_sb[:, 0:P], warm_sb[:], start=True, stop=True)
        pp_warm.release()
        pp_pro = tc.alloc_tile_pool(name="pp_pro", bufs=4, space="PSUM")

        with (
            tc.tile_pool(name="expp", bufs=40) as exppool,
            tc.tile_pool(name="smallp", bufs=6) as smallpool,
            tc.tile_pool(name="aqp", bufs=2) as aqpool,
            tc.tile_pool(name="stagep", bufs=1) as stagepool,
        ):
            e_tiles = [[None] * NK for _ in range(NHL)]  # e[h][sk]
            aq_tiles = [None] * NK  # per qtile, current pair's [q,128] tile
            stage_sb = [
                stagepool.tile([P, 512], BF16, tag=f"st{g}", name=f"st{g}")
                for g in range(16)
            ]

            # ---- emission helpers ----
            def emit_v_burst(m, pool):
                # V projection for token tile m (token-major, ones appended)
                ps = pool.tile([P, DQ], F32, tag="pj", name="pjv")
                for k in range(NK):
                    nc.tensor.matmul(
                        ps[:],
                        yt_sl[:, k, m * P : (m + 1) * P],
                        wv_sl[:, k, :],
                        start=(k == 0),
                        stop=(k == NK - 1),
                    )
                dst3 = v_sb[m][:].rearrange("p (h c) -> p h c", c=DH + 1)
                src3 = ps[:].rearrange("p (h c) -> p h c", c=DH)
                nc.vector.tensor_copy(dst3[:, :, 0:DH], src3[:, :, :])
                nc.vector.tensor_copy(
                    dst3[:, :, DH : DH + 1],
                    vones_sb[:].rearrange("p (a b) -> p a b", b=1),
                )

            def emit_proj_burst(pair, which, n, pool):
                # one [128dq, 512tok] chunk of the q/k projection for `pair`
                w_sl, src_sl, dst = (
                    (wq_sl, xt_sl, qt_sb) if which == "q" else (wk_sl, yt_sl, kt_sb)
                )
                ps = pool.tile([P, 512], F32, tag="pj", name="pj")
                for k in range(NK):
                    nc.tensor.matmul(
                        ps[:],
                        w_sl[:, k, pair * P : (pair + 1) * P],
                        src_sl[:, k, n * 512 : (n + 1) * 512],
                        start=(k == 0),
                        stop=(k == NK - 1),
                    )
                nc.vector.tensor_copy(dst[pair][:, n * 512 : (n + 1) * 512], ps[:])

            def emit_lg_exp(h, sk):
                pair, hi = divmod(h, 2)
                base = hi * DH
                lg = pp_lg.tile([P, S], F32, tag="lg", name="lg")
                for n in range(2):
                    nc.tensor.matmul(
                        lg[:, n * 512 : (n + 1) * 512],
                        kt_sb[pair][base : base + DH, sk * P : (sk + 1) * P],
                        qt_sb[pair][base : base + DH, n * 512 : (n + 1) * 512],
                        start=True,
                        stop=True,
                    )
                e = exppool.tile([P, S], MM_DT, tag="exp", name="exp")
                nc.scalar.activation(e[:], lg[:], EXP, bias=bias_sb[:, sk : sk + 1])
                return e

            def emit_av(h, qtile, pool=None):
                # attn@V for (head h, query tile qtile): psum [128q, 65]
                pair, hi = divmod(h, 2)
                av = (pool or pp_av).tile([P, DH + 1], F32, tag="av", name="av")
                for sk in range(NK):
                    nc.tensor.matmul(
                        av[:],
                        e_tiles[h][sk][:, qtile * P : (qtile + 1) * P],
                        v_sb[sk][:, h * (DH + 1) : (h + 1) * (DH + 1)],
                        start=(sk == 0),
                        stop=(sk == NK - 1),
                    )
                rc = smallpool.tile([P, 1], F32, tag="rc", name="rc")
                nc.vector.reciprocal(rc[:], av[:, DH : DH + 1])
                if hi == 0:
                    aq = aqpool.tile([P, P], MM_DT, tag=f"aq{qtile}", name=f"aq{qtile}")
                    aq_tiles[qtile] = aq
                else:
                    aq = aq_tiles[qtile]
                nc.vector.tensor_scalar_mul(
                    aq[:, hi * DH : (hi + 1) * DH], av[:, 0:DH], rc[:]
                )
                if hi == 1:
                    nc.sync.dma_start_transpose(
                        at_sb[pair][:, qtile * P : (qtile + 1) * P], aq[:]
                    )

            # ---- prologue: Q projections for ALL pairs (xt/wq arrive
            # first; PE otherwise idles in the DMA-paced window), then K for
            # pair 0 (the gate for the first logits/exp). Full-row N=1024
            # matmuls: one weight load per k-tile.
            for pr in range(4):
                for n in range(2):
                    emit_proj_burst(pr, "q", n, pp_pro)
            for n in range(2):
                emit_proj_burst(0, "k", n, pp_pro)
            pp_pro.release()
            pp_av = tc.alloc_tile_pool(name="pp_av", bufs=2, space="PSUM")
            pp_pj = tc.alloc_tile_pool(name="pp_pj", bufs=2, space="PSUM")
            pp_lg = tc.alloc_tile_pool(name="pp_lg", bufs=2, space="PSUM")

            # ---- software-pipelined head loop ----
            # Per block h: lg/exp stream (the ACT pace-setter) + fillers
            # (remaining K projections, V bursts once wv lands, lagged attn@V
            # groups), balanced so no block's PE work exceeds the ACT pace.
            fillers = {
                (0, 2): lambda: emit_proj_burst(1, "k", 0, pp_pj),
                (0, 5): lambda: emit_proj_burst(1, "k", 1, pp_pj),
                (1, 1): lambda: emit_v_burst(0, pp_pj),
                (1, 3): lambda: emit_v_burst(1, pp_pj),
                (1, 5): lambda: emit_v_burst(2, pp_pj),
                (2, 1): lambda: emit_v_burst(3, pp_pj),
                (2, 4): lambda: emit_v_burst(4, pp_pj),
                (2, 6): lambda: emit_proj_burst(2, "k", 0, pp_pj),
                (3, 1): lambda: emit_v_burst(5, pp_pj),
                (3, 3): lambda: emit_v_burst(6, pp_pj),
                (3, 5): lambda: emit_v_burst(7, pp_pj),
                (3, 7): lambda: emit_proj_burst(2, "k", 1, pp_pj),
                (4, 2): lambda: emit_proj_burst(3, "k", 0, pp_pj),
                (5, 2): lambda: emit_proj_burst(3, "k", 1, pp_pj),
            }

            av_sched = {4: [0], 5: [1], 6: [2, 3], 7: [4, 5]}
            for h in range(NHL):
                for sk in range(NK):
                    e_tiles[h][sk] = emit_lg_exp(h, sk)
                    f = fillers.get((h, sk))
                    if f is not None:
                        f()
                    for avh in av_sched.get(h, ()):
                        emit_av(avh, sk)
            for qtile in range(NK):
                emit_av(6, qtile)
            # ---- tail: av(7) drain on a deeper PSUM pool (3 groups in
            # flight), then Wo: 4-pair PSUM accumulation per output chunk,
            # psum->SBUF copies alternating DVE/ACT, DMA per chunk
            pp_lg.release()
            pp_pj.release()
            pp_av2 = tc.alloc_tile_pool(name="pp_av2", bufs=2, space="PSUM")
            pp_wo = tc.alloc_tile_pool(name="pp_wo", bufs=3, space="PSUM")

            def emit_wo(m, n):
                g = 2 * m + n
                ps = pp_wo.tile([P, 512], F32, tag="wops", name="wops")
                for pair in range(4):
                    nc.tensor.matmul(
                        ps[:],
                        at_sb[pair][:, m * P : (m + 1) * P],
                        wo_sl[:, pair : pair + 1, n * 512 : (n + 1) * 512],
                        start=(pair == 0),
                        stop=(pair == 3),
                    )
                if g % 2 == 0:
                    nc.vector.tensor_copy(stage_sb[g][:], ps[:])
                else:
                    nc.scalar.copy(stage_sb[g][:], ps[:])
                (nc.sync if g % 2 == 0 else nc.scalar).dma_start(
                    out[m * P : (m + 1) * P, n * 512 : (n + 1) * 512], stage_sb[g][:]
                )

            for qtile in range(NK):
                emit_av(NHL - 1, qtile,
                        pool=(pp_av2 if qtile % 2 == 0 else pp_av))
            for qtile in range(NK):
                emit_wo(qtile, 0)
                emit_wo(qtile, 1)
            pp_wo.release()
            pp_av2.release()
            pp_av.release()


def _prep_in_maps(x, y, bias, Wq, Wk, Wv, Wo):
    x = np.asarray(x, dtype=np.float32)
    y = np.asarray(y, dtype=np.float32)
    bias = np.asarray(bias, dtype=np.float32)
    Wq = np.asarray(Wq, dtype=np.float32)
    Wk = np.asarray(Wk, dtype=np.float32)
    Wv = np.asarray(Wv, dtype=np.float32)
    Wo = np.asarray(Wo, dtype=np.float32)
    scale = 1.0 / np.sqrt(DH)
    bf = ml_dtypes.bfloat16

    def slab(a):
        # [rows, cols] -> [128, ktile, cols]
        return np.ascontiguousarray(
            a.reshape(a.shape[0] // P, P, a.shape[1]).transpose(1, 0, 2)
        )

    in_maps = []
    for c in range(N_CORES):
        b, hf = divmod(c, 2)
        cols = slice(hf * DQ, (hf + 1) * DQ)
        in_maps.append(
            {
                "xt": slab(x[b].T.astype(bf)),
                "yt": slab(y[b].T.astype(bf)),
                "wq": slab((Wq[:, cols] * scale).astype(bf)),
                "wk": slab(Wk[:, cols].astype(bf)),
                "wv": slab(Wv[:, cols].astype(bf)),
                "wo": slab(Wo[cols, :].astype(bf)),
                "biasd": np.ascontiguousarray(bias[b, 0, 0].reshape(NK, P).T),
                "onesd": np.ones((P, NHL), dtype=bf),
            }
        )
    return in_maps


def get_program():
    global _CACHED_NC
    if _CACHED_NC is None:
        _CACHED_NC = build_program()
    return _CACHED_NC


def kernel(x, y, bias, Wq, Wk, Wv, Wo):
    nc = get_program()
    in_maps = _prep_in_maps(x, y, bias, Wq, Wk, Wv, Wo)
    res = bass_utils.run_bass_kernel_spmd(nc, in_maps, core_ids=list(range(N_CORES)))
    B = 4
    out = np.empty((B, S, HID), dtype=np.float32)
    for b in range(B):
        out[b] = res.results[2 * b]["out"].astype(np.float32) + res.results[
            2 * b + 1
        ]["out"].astype(np.float32)
    return out
